# revision 12
# baseline (speedup 1.0000x reference)
"""Trainium2 Bass kernel for nn_CustomLoss_69999376990919.

Math: the reference's A-inner-product modified Gram-Schmidt + projection
collapses to per-sample 4x4 Gram matrices
    G[s] = P_s diag(a_s) P_s^T,   R[s] = P_s diag(a_s) T_s
after which   loss = mean_s (4 - tr(R^T G^{-1} R)) / 4.
The device streams all inputs (memory-bound) and produces G/R; the tiny
4x4 solves run on the host in float64.

Sharding: pure data parallelism, batch axis 0 split across 8 cores
(64 samples each).

v10 pipeline (per core): predictions are transposed on the host to
[s, n, j] (a pure layout change, like the per-core shard slicing), so
preds and targs both stream as plain fp32 DMAs with 2 KiB descriptors at
full HBM rate.  Two DMA queues: preds+coeff on the sync HWDGE ring,
targs (and the tiny result stores) on the gpsimd SWDGE ring — the SDMA
engines round-robin between queues, so the preds ring drains ~1.8x
faster per byte and every group's W/G work completes early, leaving only
the targs-gated R work near the tail.  All PE operands are built f-MAJOR
(contiguous SBUF lines): ScalarE transpose-casts preds chunks into the
moving tile, VectorE forms W = a * P (fp32 coeff via broadcast AP) and
transpose-casts targs.  Per group of GS=16 samples: a G accumulation
chain (128 matmuls, stationary W(f), moving P(f)) runs while targs still
streams, then an R chain (moving T(f)).  64 warmup matmuls before the
last R chain keep the PE HAM clock at 2.4 GHz.  bf16 is safe: the loss
is 1 - O(1e-4).
"""

import os
from contextlib import ExitStack

import numpy as np

import concourse.bacc as bacc
import concourse.bass as bass
import concourse.tile as tile
from concourse import mybir
from concourse.bass_utils import run_bass_kernel_spmd

B, C, N = 512, 4, 16384
H = 0.0078125  # grid spacing; A = diag(h^2 * coefficients)
NCORES = 8
SPC = B // NCORES  # 64 samples per core
P = 128            # SBUF partitions; n = p*128 + f
F = N // P         # 128 f-chunks
GROUPS = [16, 16, 16, 16]  # samples per group (sum == SPC)
SC = 8             # samples per DMA/cast chunk
NWARM = 64         # PE warmup matmuls before the last R chain
OUTW = 2 * C * SPC  # 512 output columns

_CACHE = {}


def _build_bass():
    nc = bacc.Bacc(trn_type="TRN2")
    coeff = nc.dram_tensor("coeff", [SPC, N], mybir.dt.float32, kind="ExternalInput")
    # host-transposed predictions: [s, n, j]
    preds = nc.dram_tensor("preds", [SPC, N, C], mybir.dt.float32, kind="ExternalInput")
    targs = nc.dram_tensor("targs", [SPC, N, C], mybir.dt.float32, kind="ExternalInput")
    out = nc.dram_tensor("gr_out", [64, OUTW], mybir.dt.float32, kind="ExternalOutput")

    coeff_v = coeff[:].rearrange("s (p f) -> p s f", p=P)
    preds_v = preds[:].rearrange("s (p f) j -> p s f j", p=P)
    targs_v = targs[:].rearrange("s (p f) m -> p s f m", p=P)

    with tile.TileContext(nc) as tc, ExitStack() as ctx:
        a32s = ctx.enter_context(tc.tile_pool(name="a32s", bufs=2))
        p32s = ctx.enter_context(tc.tile_pool(name="p32s", bufs=2))
        t32s = ctx.enter_context(tc.tile_pool(name="t32s", bufs=2))
        m16s = ctx.enter_context(tc.tile_pool(name="m16s", bufs=2))
        w16s = ctx.enter_context(tc.tile_pool(name="w16s", bufs=2))
        outs = ctx.enter_context(tc.tile_pool(name="outs", bufs=1))
        psums = ctx.enter_context(tc.tile_pool(name="psums", bufs=3, space="PSUM"))
        psumx = ctx.enter_context(tc.tile_pool(name="psumx", bufs=1, space="PSUM"))

        out_stage = outs.tile([64, OUTW], mybir.dt.float32)
        psum_x = psumx.tile([C * GROUPS[0], C * GROUPS[0]], mybir.dt.float32)

        col = 0
        s0 = 0
        ng = len(GROUPS)
        for g, GS in enumerate(GROUPS):
            QP = C * GS          # psum partitions (s, i)
            CG = C * GS          # p-part moving cols
            MW = 2 * C * GS      # total moving cols
            nch = (GS + SC - 1) // SC

            # preds + coeff on the sync HWDGE ring
            p32 = []
            for ch in range(nch):
                c0 = s0 + ch * SC
                p32c = p32s.tile([P, SC, F, C], mybir.dt.float32, tag="p32")
                nc.sync.dma_start(out=p32c[:], in_=preds_v[:, c0 : c0 + SC, :, :])
                p32.append(p32c)
            a32 = a32s.tile([P, GS, F], mybir.dt.float32, tag="a32")
            nc.sync.dma_start(out=a32[:], in_=coeff_v[:, s0 : s0 + GS, :])
            # targs on the gpsimd SWDGE ring (separate queue)
            t32 = []
            for ch in range(nch):
                c0 = s0 + ch * SC
                t32c = t32s.tile([P, SC, F, C], mybir.dt.float32, tag="t32")
                nc.gpsimd.dma_start(out=t32c[:], in_=targs_v[:, c0 : c0 + SC, :, :])
                t32.append(t32c)

            # combined f-major moving tile: cols [0:CG] = preds (s,j),
            # cols [CG:MW] = targs (s,m)
            m16 = m16s.tile([P, F, MW], mybir.dt.bfloat16, tag="m16")
            w16f = w16s.tile([P, F, GS, C], mybir.dt.bfloat16, tag="w16f")

            for ch in range(nch):
                # preds chunk -> m16 cols (s*C + j); ScalarE (4-el runs)
                pdst = m16[:, :, ch * SC * C : (ch + 1) * SC * C].rearrange(
                    "p f (s j) -> p f s j", s=SC
                )
                nc.scalar.copy(out=pdst, in_=p32[ch][:].transpose([0, 2, 1, 3]))
            for ch in range(nch):
                sl = slice(ch * SC, (ch + 1) * SC)
                # W chunk = a * P, f-major (VectorE; fp32 a via broadcast AP)
                a_in = (
                    a32[:, sl, :]
                    .transpose([0, 2, 1])
                    .unsqueeze(3)
                    .broadcast_to([P, F, SC, C])
                )
                p_in = m16[:, :, ch * SC * C : (ch + 1) * SC * C].rearrange(
                    "p f (s j) -> p f s j", s=SC
                )
                nc.vector.tensor_mul(w16f[:, :, sl, :], a_in, p_in)
            for ch in range(nch):
                # targs chunk -> m16 cols (CG + s*C + m); VectorE
                tdst = m16[:, :, CG + ch * SC * C : CG + (ch + 1) * SC * C].rearrange(
                    "p f (s m) -> p f s m", s=SC
                )
                nc.vector.tensor_copy(tdst, t32[ch][:].transpose([0, 2, 1, 3]))

            # G chain: gated only by preds+W — runs while targs still streams
            psum_g = psums.tile([QP, CG], mybir.dt.float32, tag="pg")
            psum_r = psums.tile([QP, CG], mybir.dt.float32, tag="pr")
            for f in range(F):
                nc.tensor.matmul(
                    psum_g[:],
                    w16f[:, f, :, :],    # [128, (s, i)] stationary, dense
                    m16[:, f, 0:CG],     # [128, (s, j)] moving, dense
                    start=(f == 0),
                    stop=(f == F - 1),
                )
            if g == ng - 1:
                # keep the PE HAM clock warm across the gap before the final
                # targs-gated R chain (inputs already resident; result unused)
                for f in range(NWARM):
                    nc.tensor.matmul(
                        psum_x[:],
                        w16f[:, f, :, :],
                        m16[:, f, 0:CG],
                        start=True,
                        stop=True,
                    )
            # R chain: gated by targs casts
            for f in range(F):
                nc.tensor.matmul(
                    psum_r[:],
                    w16f[:, f, :, :],
                    m16[:, f, CG:MW],    # [128, (s, m)] moving, dense
                    start=(f == 0),
                    stop=(f == F - 1),
                )

            # psum -> staging on VectorE (keeps ScalarE FIFO free for p-casts)
            nc.vector.tensor_copy(out_stage[:QP, col : col + CG], psum_g[:])
            nc.vector.tensor_copy(out_stage[:QP, col + CG : col + MW], psum_r[:])
            # result store on the SWDGE ring (never blocks the preds ring)
            nc.gpsimd.dma_start(
                out=out[:, col : col + MW], in_=out_stage[:, col : col + MW]
            )
            col += MW
            s0 += GS

    if not nc.is_finalized():
        nc.finalize()
    return nc


def _get_nc():
    if "nc" not in _CACHE:
        _CACHE["nc"] = _build_bass()
    return _CACHE["nc"]


def kernel(coefficients, predictions, targets):
    co = np.ascontiguousarray(np.asarray(coefficients, dtype=np.float32))
    pr = np.asarray(predictions, dtype=np.float32)
    tg = np.ascontiguousarray(np.asarray(targets, dtype=np.float32))
    assert co.shape == (B, N) and pr.shape == (B, C, N) and tg.shape == (B, N, C)
    # host-side layout change: preds -> [s, n, j] so the device streams it
    # with 2 KiB DMA descriptors (same class of host work as shard slicing)
    prT = np.ascontiguousarray(pr.transpose(0, 2, 1))

    nc = _get_nc()
    in_maps = []
    for c in range(NCORES):
        sl = slice(c * SPC, (c + 1) * SPC)
        in_maps.append({"coeff": co[sl], "preds": prT[sl], "targs": tg[sl]})

    res = run_bass_kernel_spmd(nc, in_maps, core_ids=list(range(NCORES)))
    _CACHE["last"] = res

    # host epilogue: extract per-sample 4x4 G/R blocks, fp64 solve
    # psum[s*C+i, s*C+j] = G[s,i,j]; psum[s*C+i, CG + s*C+m] = R[s,i,m]
    G = np.empty((B, C, C), np.float64)
    R = np.empty((B, C, C), np.float64)
    for c in range(NCORES):
        o = np.asarray(res.results[c]["gr_out"], dtype=np.float64)
        col = 0
        s0 = 0
        for GS in GROUPS:
            QP = C * GS
            CG = C * GS
            MW = 2 * C * GS
            bg = o[:QP, col : col + CG].reshape(GS, C, GS, C)
            br = o[:QP, col + CG : col + MW].reshape(GS, C, GS, C)
            b0 = c * SPC + s0
            G[b0 : b0 + GS] = np.einsum("sisj->sij", bg)
            R[b0 : b0 + GS] = np.einsum("sism->sim", br)
            col += MW
            s0 += GS

    G = 0.5 * (G + np.swapaxes(G, 1, 2))
    Xs = np.linalg.solve(G, R)
    val = (H * H) * np.einsum("bim,bim->b", R, Xs)
    loss = np.mean((4.0 - val) / 4.0)
    return np.float32(loss)


# revision 13
# speedup vs baseline: 1.1920x; 1.1920x over previous
"""Trainium2 Bass kernel for nn_CustomLoss_69999376990919.

Math: the reference's A-inner-product modified Gram-Schmidt + projection
collapses to per-sample 4x4 Gram matrices
    G[s] = P_s diag(a_s) P_s^T,   R[s] = P_s diag(a_s) T_s
after which   loss = mean_s (4 - tr(R^T G^{-1} R)) / 4.
The device streams all inputs (memory-bound) and produces G/R; the tiny
4x4 solves run on the host in float64.

Sharding: pure data parallelism, batch axis 0 split across 8 cores
(64 samples each).

v11 pipeline (per core): predictions are transposed on the host to
[s, n, j] (a pure layout change, like the per-core shard slicing), so
preds and targs both stream as plain fp32 HWDGE DMAs on the sync ring
with 2 KiB descriptors at full HBM rate (~105 us of the ~130 us total).
All PE operands are built f-MAJOR so each f-chunk matmul reads fully
contiguous SBUF lines: ScalarE transpose-casts preds chunks into a
combined moving tile [P(f) | T(f)], VectorE transpose-casts targs and
forms W(f) = a * P(f) reading fp32 coeff via a broadcast AP.  Groups of
GS=16 samples run ONE 128-matmul accumulation chain each (stationary
W(f) [128 x 64], moving [128 x 128], dense LDWEIGHTS hidden behind the
moving phase).  The LAST group is restructured to shorten the tail: its
W/G work is prioritized (casts feed W immediately, t-casts split across
both engines) and its chain is split G-then-R, so G runs while targs
still stream and only the 128-matmul R chain (+ copy + store) remains
after the final DMA byte lands — with the PE HAM clock still warm from
the G chain.  bf16 is safe: the loss is 1 - O(1e-4).
"""

import os
from contextlib import ExitStack

import numpy as np

import concourse.bacc as bacc
import concourse.bass as bass
import concourse.tile as tile
from concourse import mybir
from concourse.bass_utils import run_bass_kernel_spmd

B, C, N = 512, 4, 16384
H = 0.0078125  # grid spacing; A = diag(h^2 * coefficients)
NCORES = 8
SPC = B // NCORES  # 64 samples per core
P = 128            # SBUF partitions; n = p*128 + f
F = N // P         # 128 f-chunks
GROUPS = [16, 16, 16, 16]  # samples per group (sum == SPC)
SC = 8             # samples per DMA/cast chunk
OUTW = 2 * C * SPC  # 512 output columns

_CACHE = {}


def _build_bass():
    nc = bacc.Bacc(trn_type="TRN2")
    coeff = nc.dram_tensor("coeff", [SPC, N], mybir.dt.float32, kind="ExternalInput")
    # host-transposed predictions: [s, n, j]
    preds = nc.dram_tensor("preds", [SPC, N, C], mybir.dt.float32, kind="ExternalInput")
    targs = nc.dram_tensor("targs", [SPC, N, C], mybir.dt.float32, kind="ExternalInput")
    out = nc.dram_tensor("gr_out", [64, OUTW], mybir.dt.float32, kind="ExternalOutput")

    coeff_v = coeff[:].rearrange("s (p f) -> p s f", p=P)
    preds_v = preds[:].rearrange("s (p f) j -> p s f j", p=P)
    targs_v = targs[:].rearrange("s (p f) m -> p s f m", p=P)

    with tile.TileContext(nc) as tc, ExitStack() as ctx:
        a32s = ctx.enter_context(tc.tile_pool(name="a32s", bufs=2))
        p32s = ctx.enter_context(tc.tile_pool(name="p32s", bufs=2))
        t32s = ctx.enter_context(tc.tile_pool(name="t32s", bufs=2))
        m16s = ctx.enter_context(tc.tile_pool(name="m16s", bufs=2))
        w16s = ctx.enter_context(tc.tile_pool(name="w16s", bufs=2))
        outs = ctx.enter_context(tc.tile_pool(name="outs", bufs=1))
        psums = ctx.enter_context(tc.tile_pool(name="psums", bufs=3, space="PSUM"))

        out_stage = outs.tile([64, OUTW], mybir.dt.float32)

        col = 0
        s0 = 0
        ng = len(GROUPS)
        for g, GS in enumerate(GROUPS):
            QP = C * GS          # psum partitions (s, i)
            CG = C * GS          # p-part moving cols
            MW = 2 * C * GS      # total moving cols
            nch = (GS + SC - 1) // SC
            last = g == ng - 1

            p32 = []
            for ch in range(nch):
                c0 = s0 + ch * SC
                p32c = p32s.tile([P, SC, F, C], mybir.dt.float32, tag="p32")
                nc.sync.dma_start(out=p32c[:], in_=preds_v[:, c0 : c0 + SC, :, :])
                p32.append(p32c)
            a32 = a32s.tile([P, GS, F], mybir.dt.float32, tag="a32")
            nc.sync.dma_start(out=a32[:], in_=coeff_v[:, s0 : s0 + GS, :])
            t32 = []
            for ch in range(nch):
                c0 = s0 + ch * SC
                t32c = t32s.tile([P, SC, F, C], mybir.dt.float32, tag="t32")
                nc.sync.dma_start(out=t32c[:], in_=targs_v[:, c0 : c0 + SC, :, :])
                t32.append(t32c)

            # combined f-major moving tile: cols [0:CG] = preds (s,j),
            # cols [CG:MW] = targs (s,m)
            m16 = m16s.tile([P, F, MW], mybir.dt.bfloat16, tag="m16")
            w16f = w16s.tile([P, F, GS, C], mybir.dt.bfloat16, tag="w16f")

            def p_cast(ch):
                pdst = m16[:, :, ch * SC * C : (ch + 1) * SC * C].rearrange(
                    "p f (s j) -> p f s j", s=SC
                )
                nc.scalar.copy(out=pdst, in_=p32[ch][:].transpose([0, 2, 1, 3]))

            def t_cast(ch, eng):
                tdst = m16[
                    :, :, CG + ch * SC * C : CG + (ch + 1) * SC * C
                ].rearrange("p f (s m) -> p f s m", s=SC)
                src = t32[ch][:].transpose([0, 2, 1, 3])
                if eng == "act":
                    nc.scalar.copy(out=tdst, in_=src)
                else:
                    nc.vector.tensor_copy(tdst, src)

            def w_mul(ch):
                sl = slice(ch * SC, (ch + 1) * SC)
                a_in = (
                    a32[:, sl, :]
                    .transpose([0, 2, 1])
                    .unsqueeze(3)
                    .broadcast_to([P, F, SC, C])
                )
                p_in = m16[:, :, ch * SC * C : (ch + 1) * SC * C].rearrange(
                    "p f (s j) -> p f s j", s=SC
                )
                nc.vector.tensor_mul(w16f[:, :, sl, :], a_in, p_in)

            if not last:
                # steady-state order (keeps every FIFO flowing)
                for ch in range(nch):
                    p_cast(ch)
                    t_cast(ch, "vec")
                    w_mul(ch)
            else:
                # tail-critical order: W path first so the G chain can run
                # while targs still stream; t-casts split across engines
                for ch in range(nch):
                    p_cast(ch)
                    w_mul(ch)
                t_cast(0, "act")
                t_cast(1, "vec")

            if not last:
                psum = psums.tile([QP, MW], mybir.dt.float32, tag="ps")
                for f in range(F):
                    nc.tensor.matmul(
                        psum[:],
                        w16f[:, f, :, :],  # [128, (s, i)] stationary, dense
                        m16[:, f, :],      # [128, (s,j | s,m)] moving, dense
                        start=(f == 0),
                        stop=(f == F - 1),
                    )
                nc.scalar.copy(out=out_stage[:QP, col : col + MW], in_=psum[:])
            else:
                psum_g = psums.tile([QP, CG], mybir.dt.float32, tag="ps")
                psum_r = psums.tile([QP, CG], mybir.dt.float32, tag="ps2")
                for f in range(F):
                    nc.tensor.matmul(
                        psum_g[:],
                        w16f[:, f, :, :],
                        m16[:, f, 0:CG],
                        start=(f == 0),
                        stop=(f == F - 1),
                    )
                for f in range(F):
                    nc.tensor.matmul(
                        psum_r[:],
                        w16f[:, f, :, :],
                        m16[:, f, CG:MW],
                        start=(f == 0),
                        stop=(f == F - 1),
                    )
                nc.scalar.copy(out=out_stage[:QP, col : col + CG], in_=psum_g[:])
                nc.scalar.copy(
                    out=out_stage[:QP, col + CG : col + MW], in_=psum_r[:]
                )

            # store on the ACT HWDGE ring: never blocks the input (sync) ring
            nc.scalar.dma_start(
                out=out[:, col : col + MW], in_=out_stage[:, col : col + MW]
            )
            col += MW
            s0 += GS

    if not nc.is_finalized():
        nc.finalize()
    return nc


def _get_nc():
    if "nc" not in _CACHE:
        _CACHE["nc"] = _build_bass()
    return _CACHE["nc"]


def kernel(coefficients, predictions, targets):
    co = np.ascontiguousarray(np.asarray(coefficients, dtype=np.float32))
    pr = np.asarray(predictions, dtype=np.float32)
    tg = np.ascontiguousarray(np.asarray(targets, dtype=np.float32))
    assert co.shape == (B, N) and pr.shape == (B, C, N) and tg.shape == (B, N, C)
    # host-side layout change: preds -> [s, n, j] so the device streams it
    # with 2 KiB DMA descriptors (same class of host work as shard slicing)
    prT = np.ascontiguousarray(pr.transpose(0, 2, 1))

    nc = _get_nc()
    in_maps = []
    for c in range(NCORES):
        sl = slice(c * SPC, (c + 1) * SPC)
        in_maps.append({"coeff": co[sl], "preds": prT[sl], "targs": tg[sl]})

    res = run_bass_kernel_spmd(nc, in_maps, core_ids=list(range(NCORES)))
    _CACHE["last"] = res

    # host epilogue: extract per-sample 4x4 G/R blocks, fp64 solve
    # psum[s*C+i, s*C+j] = G[s,i,j]; psum[s*C+i, CG + s*C+m] = R[s,i,m]
    G = np.empty((B, C, C), np.float64)
    R = np.empty((B, C, C), np.float64)
    for c in range(NCORES):
        o = np.asarray(res.results[c]["gr_out"], dtype=np.float64)
        col = 0
        s0 = 0
        for GS in GROUPS:
            QP = C * GS
            CG = C * GS
            MW = 2 * C * GS
            bg = o[:QP, col : col + CG].reshape(GS, C, GS, C)
            br = o[:QP, col + CG : col + MW].reshape(GS, C, GS, C)
            b0 = c * SPC + s0
            G[b0 : b0 + GS] = np.einsum("sisj->sij", bg)
            R[b0 : b0 + GS] = np.einsum("sism->sim", br)
            col += MW
            s0 += GS

    G = 0.5 * (G + np.swapaxes(G, 1, 2))
    Xs = np.linalg.solve(G, R)
    val = (H * H) * np.einsum("bim,bim->b", R, Xs)
    loss = np.mean((4.0 - val) / 4.0)
    return np.float32(loss)
